# revision 27
# baseline (speedup 1.0000x reference)
"""Self-contained Trainium2 Bass kernel for nn_AtomsNetwork (gnn_message_passing).

Sharding: atoms split 8 ways across the chip's NeuronCores (2000/core).
Per protein p:
  L1: neighbor-signal tables sig_s=atoms@Wsr1, sig_d=atoms@Wdr1 are built
      shard-wise on TensorE in row-major layout (bf16), AllGathered via the
      collective engine into DRAM. Neighbor rows are then gathered straight
      from the DRAM table with non-transposing SWDGE dma_gather chunks that
      round-robin over 4 SWDGE queues (descriptor generation runs on the Q7
      core pair (2q, 2q+1), so 4 queues give ~3x desc-gen throughput; the
      non-transpose path avoids the single shared Xbar so concurrent queues
      are safe). Gathered blocks land position-major [128 pos, 128 feat];
      DVE scales them by per-position 1/cnt (pad positions scaled by 0),
      and TensorE one-hot matmuls segment-sum the K=10 windows directly
      into the zpsum accumulation alongside atoms@Wv and the streamed
      residues@Wr matmuls; y = relu(zpsum).
  L2: y itself is the table (row-major bf16, AllGathered); means are
      segment-summed into per-side PSUM tiles, copied to SBUF, and applied
      through Wsr2/Wdr2 matmuls into the w accumulation; w = relu(...).
Residue means: one-hot segment matmul per 128-atom chunk, partial sums
AllReduced across cores; r1's 1/count is folded into the host-built row
`sel` matrix, r2's applied via a ones-outer-product broadcast multiply.
Head: each core computes its 50 rows of the 400x400 residue-pair grid;
x@Wf1 is decomposed as A[i]+B[j] (rank trick), relu'd via per-partition
bias activation ops, then Wf2/Wf3 matmuls.
Pool-queue order keeps all independent gathers ahead of each collective's
semaphore wait so AllGathers/AllReduces never stall later gather descgen.
"""
import sys
import numpy as np

sys.path.insert(0, '/opt/trn_rl_repo')

N_ATOMS = 16000
NC = 8
K = 10
N_RES = 400
ATOM_CAT = 12
BERT_DIM = 1024
DF2 = 64


def build_graph(N, R, BERT):
    from concourse import bass, bacc, mybir
    from concourse.alu_op_type import AluOpType
    f32, bf16, i16 = mybir.dt.float32, mybir.dt.bfloat16, mybir.dt.int16
    AF = mybir.ActivationFunctionType

    LOC = N // NC
    LOCP = -(-LOC // 128) * 128
    STR = LOCP // 128 + (1 if LOC == LOCP else 0)
    CH_AT = 256                # atoms per gather chunk
    CH = CH_AT * K             # indices per gather chunk
    BL = CH // 128             # gathered position blocks per chunk
    NSEG = LOCP // CH_AT       # chunks per side
    NID = 2 * LOCP * K
    KB = BERT // 128
    MYR = R // NC
    RC = -(-R // 128)
    AC = LOCP // 128           # atom chunks of 128
    NTRIM = CH - (LOCP - LOC) * K   # valid idxs in each side's tail chunk
    RPC = 2 * NSEG * BL        # recipPos columns per side... total per protein

    nc = bacc.Bacc(num_swdge_queues=4)
    P = lambda n, s, d: nc.declare_dram_parameter(n, s, d, isOutput=False)
    ins = {}
    for p in (1, 2):
        ins[f'atomsT_{p}'] = P(f'atomsT_{p}', [ATOM_CAT, LOCP], bf16)
        ins[f'residT_{p}'] = P(f'residT_{p}', [KB, 128, LOCP], bf16)
        ins[f'idxL1_{p}'] = P(f'idxL1_{p}', [128, NID // 16], i16)
        ins[f'idxL2_{p}'] = P(f'idxL2_{p}', [128, NID // 16], i16)
        ins[f'recipPos_{p}'] = P(f'recipPos_{p}', [128, RPC], bf16)
        ins[f'rids_{p}'] = P(f'rids_{p}', [128, AC], f32)
    for nm, sh in [('Wsv', [128, 128]),
                   ('Wf1t', [128, 256]), ('Wf1b', [128, 256]),
                   ('bf1', [128, 2]), ('bf2', [DF2, 1]), ('bf3', [1, 1]),
                   ('recip_res', [1, 2 * R])]:
        ins[nm] = P(nm, sh, f32)
    ins['Wf2'] = P('Wf2', [128, 2 * DF2], bf16)
    ins['sel'] = P('sel', [128, RC * MYR], bf16)
    ins['Wf3'] = P('Wf3', [DF2, 1], bf16)
    ins['Wr'] = P('Wr', [128, KB * 128], bf16)
    ins['oneH'] = P('oneH', [128, K * 128], bf16)
    for nm in ('Wv', 'Wsr1', 'Wdr1'):
        ins[nm] = P(nm, [ATOM_CAT, 128], bf16)
    for nm in ('Wsr2', 'Wdr2'):
        ins[nm] = P(nm, [128, 128], f32)
    out_ext = nc.declare_dram_parameter('out', [1, MYR * R], f32, isOutput=True)
    import os
    DBG = bool(os.environ.get('KDBG'))
    dbg_ext = nc.declare_dram_parameter('dbg', [128, 2 * R + R + MYR + 2 * MYR + 4 * 128], f32, isOutput=True) if DBG else None
    dbg2_ext = nc.declare_dram_parameter('dbg2', [128, 4496], f32, isOutput=True) if DBG else None

    shardL1s = [nc.dram_tensor(f'shardL1_{p}', [2, STR, 128, 128], bf16) for p in (0, 1)]
    fullL1s = [nc.dram_tensor(f'fullL1_{p}', [NC, 2, STR, 128, 128], bf16, addr_space='Shared')
               for p in (0, 1)]
    shardYs = [nc.dram_tensor(f'shardY{p}', [1, STR, 128, 128], bf16) for p in (0, 1)]
    fullY = [nc.dram_tensor(f'fullY{p}', [NC, 1, STR, 128, 128], bf16, addr_space='Shared')
             for p in (0, 1)]
    rparts_d = [nc.dram_tensor(f'rpart_d{p}', [128, R], f32) for p in (0, 1)]
    rsums_d = [nc.dram_tensor(f'rsum_d{p}', [128, R], f32, addr_space='Shared')
               for p in (0, 1)]

    steps = []
    cnt = {}

    class Tok:
        __slots__ = ('sem', 'n')
        def __init__(s, sem, n): s.sem, s.n = sem, n

    from contextlib import ExitStack
    _es = ExitStack()
    with _es:
        block = _es.enter_context(nc.Block())
        sem_dma = _es.enter_context(nc.semaphore('dma'))
        sem_gat0 = _es.enter_context(nc.semaphore('gat0'))
        sem_gat1 = _es.enter_context(nc.semaphore('gat1'))
        sem_gat2 = _es.enter_context(nc.semaphore('gat2'))
        sem_gat3 = _es.enter_context(nc.semaphore('gat3'))
        sem_dmaS = _es.enter_context(nc.semaphore('dmaS'))
        sem_dmaT = _es.enter_context(nc.semaphore('dmaT'))
        sem_dmaR0 = _es.enter_context(nc.semaphore('dmaR0'))
        sem_dmaR1 = _es.enter_context(nc.semaphore('dmaR1'))
        sem_dmaU = _es.enter_context(nc.semaphore('dmaU'))
        sem_dmaS2 = _es.enter_context(nc.semaphore('dmaS2'))
        sem_dmaD1 = _es.enter_context(nc.semaphore('dmaD1'))
        sem_dmaD2 = _es.enter_context(nc.semaphore('dmaD2'))
        sem_dmaO2 = _es.enter_context(nc.semaphore('dmaO2'))
        sem_dmaO3 = _es.enter_context(nc.semaphore('dmaO3'))
        sem_pe = _es.enter_context(nc.semaphore('pe'))
        sem_v = _es.enter_context(nc.semaphore('v'))
        sem_act = _es.enter_context(nc.semaphore('act'))
        sem_g = _es.enter_context(nc.semaphore('g'))
        sem_cc = _es.enter_context(nc.semaphore('cc'))
        gbuf = _es.enter_context(nc.sbuf_tensor('gbuf', [128, 8, BL, 128], bf16))
        gsc = _es.enter_context(nc.sbuf_tensor('gsc', [128, 10, BL, 128], bf16))
        idxb = [_es.enter_context(nc.sbuf_tensor(f'idx{i}', [128, NID // 16], i16))
                for i in range(4)]
        oneHb = _es.enter_context(nc.sbuf_tensor('oneHb', [128, K * 128], bf16))
        recipb = _es.enter_context(nc.sbuf_tensor('recipb', [128, 2, RPC], bf16))
        msb = _es.enter_context(nc.sbuf_tensor('msb', [128, 2, 128], f32))
        ybuf = _es.enter_context(nc.sbuf_tensor('ybuf', [128, 2 * LOCP], f32))
        wbuf = _es.enter_context(nc.sbuf_tensor('wbuf', [128, LOCP], f32))
        rows16 = _es.enter_context(nc.sbuf_tensor('rows16', [128, 2, 128], bf16))
        rowsR = _es.enter_context(nc.sbuf_tensor('rowsR', [128, RC, 128], bf16))
        stripes = _es.enter_context(nc.sbuf_tensor('stripes', [128, 2, STR, 128], bf16))
        resb = _es.enter_context(nc.sbuf_tensor('resb', [128, 2 * LOCP], bf16))
        Mbuf = _es.enter_context(nc.sbuf_tensor('Mbuf', [128, 2 * R], bf16))
        atomsT = _es.enter_context(nc.sbuf_tensor('atomsT', [ATOM_CAT, 2 * LOCP], bf16))
        ridsb = _es.enter_context(nc.sbuf_tensor('ridsb', [128, 2 * AC], f32))
        rbuf = _es.enter_context(nc.sbuf_tensor('rbuf', [128, 2 * R], f32))
        rT = _es.enter_context(nc.sbuf_tensor('rT', [128, R], f32))
        r1my = _es.enter_context(nc.sbuf_tensor('r1my', [128, MYR], f32))
        Abuf = _es.enter_context(nc.sbuf_tensor('Abuf', [128, 2 * MYR], f32))
        Bbuf = _es.enter_context(nc.sbuf_tensor('Bbuf', [128, 2 * R], bf16))
        Xbuf = _es.enter_context(nc.sbuf_tensor('Xbuf', [128, 2, 4 * R], bf16))
        h2b = _es.enter_context(nc.sbuf_tensor('h2b', [DF2, 2, 2 * R], bf16))
        outb = _es.enter_context(nc.sbuf_tensor('outb', [1, 4, 2 * R], f32))
        iotaP = _es.enter_context(nc.sbuf_tensor('iotaP', [128, 128], f32))
        iotaR = _es.enter_context(nc.sbuf_tensor('iotaR', [128, R], f32))
        ones1 = _es.enter_context(nc.sbuf_tensor('ones1', [1, 128], f32))
        wWv = _es.enter_context(nc.sbuf_tensor('wWv', [ATOM_CAT, 128], bf16))
        wWr = _es.enter_context(nc.sbuf_tensor('wWr', [128, KB * 128], bf16))
        wWsr1 = _es.enter_context(nc.sbuf_tensor('wWsr1', [ATOM_CAT, 128], bf16))
        wWdr1 = _es.enter_context(nc.sbuf_tensor('wWdr1', [ATOM_CAT, 128], bf16))
        wWsv = _es.enter_context(nc.sbuf_tensor('wWsv', [128, 128], f32))
        wWsr2 = _es.enter_context(nc.sbuf_tensor('wWsr2', [128, 128], f32))
        wWdr2 = _es.enter_context(nc.sbuf_tensor('wWdr2', [128, 128], f32))
        wWf1t = _es.enter_context(nc.sbuf_tensor('wWf1t', [128, 256], f32))
        wWf1b = _es.enter_context(nc.sbuf_tensor('wWf1b', [128, 256], f32))
        wWf2 = _es.enter_context(nc.sbuf_tensor('wWf2', [128, 2 * DF2], bf16))
        wWf3 = _es.enter_context(nc.sbuf_tensor('wWf3', [DF2, 1], bf16))
        wbf1 = _es.enter_context(nc.sbuf_tensor('wbf1', [128, 2], f32))
        wbf2 = _es.enter_context(nc.sbuf_tensor('wbf2', [DF2, 1], f32))
        wbf3 = _es.enter_context(nc.sbuf_tensor('wbf3', [1, 1], f32))
        wrr = _es.enter_context(nc.sbuf_tensor('wrr', [1, R], f32))
        wsel = _es.enter_context(nc.sbuf_tensor('wsel', [128, RC * MYR], bf16))
        sems = {'dma': sem_dma, 'pe': sem_pe, 'v': sem_v,
                'act': sem_act, 'g': sem_g, 'cc': sem_cc,
                'gat0': sem_gat0, 'gat1': sem_gat1, 'gat2': sem_gat2, 'gat3': sem_gat3,
                'dmaS': sem_dmaS, 'dmaT': sem_dmaT, 'dmaR0': sem_dmaR0,
                'dmaR1': sem_dmaR1, 'dmaU': sem_dmaU, 'dmaS2': sem_dmaS2,
                'dmaD1': sem_dmaD1, 'dmaD2': sem_dmaD2,
                'dmaO2': sem_dmaO2, 'dmaO3': sem_dmaO3}

        def S(eng, emit, waits=(), inc=None, amt=1):
            _m = {}
            for t in waits:
                if t is not None and _m.get(id(t.sem), (None, -1))[1] < t.n:
                    _m[id(t.sem)] = (t.sem, t.n)
            cw = list(_m.values())
            semobj = sems[inc] if inc else None
            def fn(e, cw=cw, emit=emit, semobj=semobj, amt=amt):
                for sm, n in cw:
                    e.wait_ge(sm, n)
                r = emit(e)
                if semobj is not None:
                    r.then_inc(semobj, amt)
            steps.append((eng, fn))
            if inc:
                cnt[inc] = cnt.get(inc, 0) + amt
                return Tok(sems[inc], cnt[inc])
            return None

        zpsum = nc.place_psum_tensor('zps', [128, LOCP], f32, bank=0)
        trps = [nc.place_psum_tensor(f'tr{i}', [128, 128], f32, bank=i) for i in (0, 1)]
        mps = [nc.place_psum_tensor(f'mp{i}', [128, 128], f32, bank=4 + i) for i in (0, 1)]
        segps = nc.place_psum_tensor('seg', [128, R], f32, bank=2)
        r1ps = nc.place_psum_tensor('r1p', [128, MYR], f32, bank=6)
        rsp = nc.place_psum_tensor('rsp', [128, 128], f32, bank=7)
        rrps = nc.place_psum_tensor('rrp', [128, R], f32, bank=4)
        Bps = [nc.place_psum_tensor(f'Bp{i}', [128, R], f32, bank=6 + i) for i in (0, 1)]
        h2ps = [nc.place_psum_tensor(f'h2p{i}', [DF2, 2, 512], f32, bank=2 * i) for i in (0, 1)]
        h3ps = [nc.place_psum_tensor(f'h3p{i}', [1, 2, 512], f32, bank=4 + 2 * i) for i in (0, 1)]

        D = lambda out, in_: (lambda e: e.dma_start(out=out, in_=in_))

        # ---------- phase 0: constants + input loads ----------
        t_dma = None
        for nm, dst in [('Wv', wWv), ('Wr', wWr), ('Wsr1', wWsr1), ('Wdr1', wWdr1),
                        ('Wsv', wWsv), ('Wsr2', wWsr2), ('Wdr2', wWdr2),
                        ('Wf1t', wWf1t), ('Wf1b', wWf1b), ('Wf2', wWf2),
                        ('Wf3', wWf3), ('bf1', wbf1), ('bf2', wbf2), ('bf3', wbf3),
                        ('sel', wsel), ('oneH', oneHb)]:
            t_dma = S('sync', D(dst[:], ins[nm][:]), inc='dma', amt=16)
        t_dma = S('sync', D(wrr[:], ins['recip_res'][:, R:2 * R]), inc='dma', amt=16)
        for p in (1, 2):
            t_dma = S('sync', D(atomsT[:, (p - 1) * LOCP:p * LOCP], ins[f'atomsT_{p}'][:]),
                      inc='dma', amt=16)
            t_dma = S('sync', D(ridsb[:, (p - 1) * AC:p * AC], ins[f'rids_{p}'][:]),
                      inc='dma', amt=16)
            t_dma = S('sync', D(recipb[:, p - 1, :], ins[f'recipPos_{p}'][:]),
                      inc='dma', amt=16)
        for i, nm in enumerate(('idxL1_1', 'idxL1_2', 'idxL2_1', 'idxL2_2')):
            tlast = S('sync', D(idxb[i][:], ins[nm][:]), inc='dmaT', amt=16)
        # DMA completions on one semaphore are unordered; wait for all four.
        t_idx = [tlast] * 4

        t_io = S('g', lambda e: e.iota(iotaP[:], [[1, 128]], channel_multiplier=-1,
                                       allow_small_or_imprecise_dtypes=True), inc='g')
        t_id = S('v', lambda e: e.tensor_scalar(out=iotaP[:], in0=iotaP[:], scalar1=0.0,
                                                scalar2=None, op0=AluOpType.is_equal),
                 waits=[t_io], inc='v')
        t_ir = S('g', lambda e: e.iota(iotaR[:], [[1, R]], channel_multiplier=0,
                                       allow_small_or_imprecise_dtypes=True),
                 waits=[t_io], inc='g')
        t_ones = S('v', lambda e: e.memset(ones1[:], 1.0), inc='v')
        # gather-trimmed tail positions are read (scaled) before ever being
        # written; they must hold finite bf16 data.
        t_gz = S('v', lambda e: e.memset(gbuf[:].rearrange('p a b e -> p (a b e)'), 0.0),
                 inc='v')

        state = {'slot_scale': [None] * 8,   # frees gather slot (v)
                 'slot_mean': [None] * 10,                 # frees gsc slot (pe)
                 'stripes_free': None, 'tr': [None, None],
                 'mp_copy': [None, None], 'gc': 0}

        def gathers_means(idx_i, tok_cc, dram_rows, pnum, layer,
                          zp_start_waits, apply_W=None):
            """Emit gathers ('g') + scales ('v') + per-tile zpsum matmuls ('pe').

            layer 0: means accumulate directly into zpsum (table includes Wsr1/Wdr1).
            layer 1: per-side segsum into mps tile -> Act copy to msb -> apply
                     through wWsr2/wWdr2 into zpsum.
            Returns (last_pe_token, list of per-chunk scale tokens).
            Emits the start (Wv/Wsv) matmuls first: caller passes them via
            zp_start_waits = (emit_start_fn, waits).
            """
            emit_start, start_waits = zp_start_waits
            t_start = emit_start(start_waits)
            last_pe = t_start
            for c in range(2 * NSEG):
                side, q = c // NSEG, c % NSEG
                gc = state['gc']; state['gc'] = gc + 1
                buf, qn, sc = gc % 8, gc % 4, gc % 10
                nreg = CH
                tg = S('g', (lambda e, c=c, buf=buf, qn=qn, idx_i=idx_i, dram_rows=dram_rows, nreg=nreg:
                             e.dma_gather(
                                 out_ap=gbuf[:, buf, :, :],
                                 in_ap=dram_rows,
                                 idxs_ap=idxb[idx_i][:, c * (CH // 16):(c + 1) * (CH // 16)],
                                 num_idxs=CH, num_idxs_reg=nreg,
                                 elem_size=128, transpose=False,
                                 single_packet=False,
                                 queue_num=qn)),
                        waits=[tok_cc, t_idx[idx_i],
                               state['slot_scale'][buf] or t_gz],
                        inc=('gat0', 'gat1', 'gat2', 'gat3')[qn], amt=16)
                c0 = (side * NSEG + q) * BL
                tsc = S('v', (lambda e, buf=buf, sc=sc, pnum=pnum, c0=c0:
                              e.tensor_tensor(
                                  out=gsc[:, sc, :, :],
                                  in0=gbuf[:, buf, :, :],
                                  in1=recipb[:, pnum - 1, c0:c0 + BL]
                                      .unsqueeze(2).to_broadcast([128, BL, 128]),
                                  op=AluOpType.mult)),
                         waits=[tg, t_dma, state['slot_mean'][sc]], inc='v')
                state['slot_scale'][buf] = tsc
                # per-tile matmuls (PE, in-order)
                for t in (0, 1):
                    base = q * CH_AT + t * 128
                    if layer == 0:
                        for j in range(K):
                            last_pe = S('pe', (lambda e, sc=sc, t=t, j=j, base=base:
                                               e.matmul(zpsum[:, base:base + 128],
                                                        gsc[:, sc, t * K + j, :],
                                                        oneHb[:, j * 128:(j + 1) * 128],
                                                        start=False, stop=False)),
                                        waits=[tsc, t_dma], inc='pe')
                    else:
                        mp = mps[t]
                        for j in range(K):
                            last_pe = S('pe', (lambda e, sc=sc, t=t, j=j, mp=mp:
                                               e.matmul(mp[:], gsc[:, sc, t * K + j, :],
                                                        oneHb[:, j * 128:(j + 1) * 128],
                                                        start=(j == 0), stop=(j == K - 1))),
                                        waits=[tsc, t_dma, state['mp_copy'][t]], inc='pe')
                        tcp = S('act', (lambda e, t=t, mp=mp:
                                        e.activation(msb[:, t, :], mp[:], AF.Copy)),
                                waits=[last_pe], inc='act')
                        state['mp_copy'][t] = tcp
                        W = wWsr2 if side == 0 else wWdr2
                        # one stop per 512B zero region: its last writer is
                        # side1's apply of the odd chunk's second tile.
                        stp = (side == 1 and q % 2 == 1 and t == 1)
                        last_pe = S('pe', (lambda e, t=t, base=base, W=W, stp=stp:
                                           e.matmul(zpsum[:, base:base + 128],
                                                    W[:], msb[:, t, :],
                                                    start=False, stop=stp)),
                                    waits=[tcp, t_dma], inc='pe')
                state['slot_mean'][sc] = last_pe
            return last_pe

        def emit_rows(src_ap_fn, n_chunks, dst_copy_fn, tok_src, copy_eng='v',
                      extra_first_wait=None):
            """PE-transpose n_chunks [128,128] blocks of src, copy each out."""
            toks = []
            for c in range(n_chunks):
                tps = trps[c % 2]
                w = [tok_src, t_id, state['tr'][c % 2]]
                if c == 0 and extra_first_wait is not None:
                    w.append(extra_first_wait)
                tk = S('pe', (lambda e, tps=tps, c=c:
                              (lambda sap: e.transpose(tps[0:sap.shape[-1], :], sap,
                                                       iotaP[:]))(src_ap_fn(c))),
                       waits=w, inc='pe')
                if copy_eng == 'v':
                    tc = S('v', (lambda e, tps=tps, c=c: dst_copy_fn(e, tps, c)),
                           waits=[tk], inc='v')
                else:
                    tc = S('act', (lambda e, tps=tps, c=c: dst_copy_fn(e, tps, c)),
                           waits=[tk], inc='act')
                state['tr'][c % 2] = tc
                toks.append(tc)
            return toks

        # ---------- sig tables: build stripes + upload (both proteins) ----------
        tshs = []
        for p in (0, 1):
            aT0 = p * LOCP
            toks_stripe = []
            for tab, W in ((0, wWsr1), (1, wWdr1)):
                for c in range(AC):
                    buf = (tab * AC + c) % 2
                    tps = trps[buf]
                    w = [t_dma, state['tr'][buf]]
                    if len(toks_stripe) == 0 and state['stripes_free'] is not None:
                        w.append(state['stripes_free'])
                    tk = S('pe', (lambda e, tps=tps, c=c, W=W, aT0=aT0:
                                  e.matmul(tps[:], atomsT[:, aT0 + c * 128:aT0 + (c + 1) * 128],
                                           W[:], start=True, stop=True)),
                           waits=w, inc='pe')
                    tc = S('act', (lambda e, tps=tps, tab=tab, c=c:
                                   e.activation(stripes[:, tab, c, :], tps[:], AF.Copy)),
                           waits=[tk] + ([state['stripes_free']]
                                         if state['stripes_free'] else []),
                           inc='act')
                    state['tr'][buf] = tc
                    toks_stripe.append(tc)
            if STR > AC:
                toks_stripe.append(S('v', (lambda e: e.memset(stripes[:, 0, STR - 1, :], 0.0)),
                                     waits=[toks_stripe[-1]], inc='v'))
                toks_stripe.append(S('v', (lambda e: e.memset(stripes[:, 1, STR - 1, :], 0.0)),
                                     waits=[toks_stripe[-1]], inc='v'))
            tsh = S('sync', (lambda e, p=p: e.dma_start(
                        out=shardL1s[p][:].rearrange('t s p e -> p t s e'),
                        in_=stripes[:])),
                    waits=[toks_stripe[-1]], inc=('dmaS', 'dmaS2')[p], amt=16)
            tshs.append(tsh)
            state['stripes_free'] = tsh

        tabL1rows = [fullL1s[p][:].rearrange('r t s p e -> (r t s p) e') for p in (0, 1)]
        tabL1rows0_settle = tabL1rows[0]
        stripe_tr = [t for t in state['tr'] if t]
        sig_cc = [None, None]
        sig_cc[0] = S('g', (lambda e: e.collective_compute(
                    'AllGather', mybir.AluOpType.bypass,
                    replica_groups=[list(range(NC))],
                    ins=[shardL1s[0][:]], outs=[fullL1s[0][:]])),
                waits=[tshs[0]], inc='cc')
        # settle chain: a few chained DMA round-trips between the AllGather's
        # completion semaphore and the first gather reads of fullL1s[0]; the
        # completion sem can race the final collective writes under load.
        t_settle = sig_cc[0]
        for _s in range(3):
            t_settle = S('sync', (lambda e, _s=_s: e.dma_start(
                        out=Bbuf[0:16, _s * 16:(_s + 1) * 16],
                        in_=tabL1rows0_settle[_s * 16:(_s + 1) * 16, 0:16])),
                    waits=[t_settle], inc='dmaU', amt=16)
        sig_cc0_settled = t_settle
        sig_cc[1] = S('g', (lambda e: e.collective_compute(
                    'AllGather', mybir.AluOpType.bypass,
                    replica_groups=[list(range(NC))],
                    ins=[shardL1s[1][:]], outs=[fullL1s[1][:]])),
                waits=[tshs[1]], inc='cc')

        # ---------- L1 ----------
        tabYrows = [fullY[p][:].rearrange('r t s p e -> (r t s p) e') for p in (0, 1)]

        tokens_y = []
        y_cc = []
        rows_tok_y = [None, None]
        for p in (0, 1):
            aT0 = p * LOCP

            def mk_start_L1(waits, p=p, aT0=aT0):
                tok = None
                for nt in range(AC // 4):
                    n0, n1 = nt * 512, (nt + 1) * 512
                    tok = S('pe', (lambda e, n0=n0, n1=n1, aT0=aT0:
                                   e.matmul(zpsum[:, n0:n1], wWv[:],
                                            atomsT[:, aT0 + n0:aT0 + n1],
                                            start=True, stop=False)),
                            waits=[t_dma] + waits, inc='pe')
                return tok

            zw = list(stripe_tr)
            if p == 1:
                zw = [tokens_y[0], rows_tok_y[0]]
            gathers_means(p, sig_cc0_settled if p == 0 else sig_cc[p],
                          tabL1rows[p], p + 1, 0,
                          (mk_start_L1, zw))

            tres_prev = [None, None]
            lmm = None
            for kb in range(KB):
                buf = kb % 2
                trd = S('sync', D(resb[:, buf * LOCP:(buf + 1) * LOCP],
                                  ins[f'residT_{p + 1}'][kb]),
                        waits=[tres_prev[0]], inc=f'dmaR{buf}', amt=16)
                for nt in range(AC // 4):
                    n0, n1 = nt * 512, (nt + 1) * 512
                    lmm = S('pe', (lambda e, kb=kb, n0=n0, n1=n1, buf=buf:
                                   e.matmul(zpsum[:, n0:n1],
                                            wWr[:, kb * 128:(kb + 1) * 128],
                                            resb[:, buf * LOCP + n0:buf * LOCP + n1],
                                            start=False, stop=(kb == KB - 1))),
                            waits=[trd, t_dma], inc='pe')
                tres_prev = [tres_prev[1], lmm]
            t_y = S('act', (lambda e, p=p: e.activation(
                        ybuf[:, p * LOCP:(p + 1) * LOCP], zpsum[:], AF.Relu)),
                    waits=[lmm], inc='act')
            tokens_y.append(t_y)

            # --- y rows -> stripes[:,0] -> AllGather ---
            rows_toks = emit_rows(
                (lambda c, p=p: ybuf[:, p * LOCP + c * 128:p * LOCP + (c + 1) * 128]),
                AC,
                (lambda e, tps, c: e.activation(stripes[:, 0, c, :], tps[:], AF.Copy)),
                t_y, copy_eng='act', extra_first_wait=state['stripes_free'])
            rows_tok_y[p] = rows_toks[-1]
            tz = rows_toks[-1]
            if STR > AC:
                tz = S('v', (lambda e: e.memset(stripes[:, 0, STR - 1, :], 0.0)),
                       waits=[tz], inc='v')
            tshy = S('sync', D(shardYs[p][:].rearrange('t s p e -> p t s e'),
                               stripes[:, 0:1, :, :]),
                     waits=[tz], inc=('dmaS', 'dmaS2')[p], amt=16)
            state['stripes_free'] = tshy
            y_cc.append(('pending', p, tshy))

        # ---------- L2 ----------
        t_r = [None, None]
        seg_state = {'last_segs': []}

        def emit_seg_chunks(p, t_w):
            """w rows transpose + residue one-hot seg matmuls for protein p."""
            tseg = None
            segs = []
            for c in range(AC):
                mb = c % 2
                tpsb = trps[mb]
                wtr = [t_w, t_id, state['tr'][mb]]
                if len(segs) >= 2:
                    wtr.append(segs[-2])
                if c == 0 and seg_state['last_segs']:
                    wtr += seg_state['last_segs']
                tk = S('pe', (lambda e, tpsb=tpsb, c=c:
                              e.transpose(tpsb[:], wbuf[:, c * 128:(c + 1) * 128],
                                          iotaP[:])),
                       waits=wtr, inc='pe')
                trow = S('act', (lambda e, tpsb=tpsb, mb=mb:
                                 e.activation(rows16[:, mb, :], tpsb[:], AF.Copy)),
                         waits=[tk] + ([segs[-2]] if len(segs) >= 2 else []), inc='act')
                state['tr'][mb] = trow
                tM = S('v', (lambda e, c=c, p=p, mb=mb:
                             e.tensor_scalar(out=Mbuf[:, mb * R:(mb + 1) * R],
                                             in0=iotaR[:],
                                             scalar1=ridsb[:, p * AC + c:p * AC + c + 1],
                                             scalar2=None,
                                             op0=AluOpType.is_equal)),
                       waits=[t_ir, t_dma] + ([segs[-2]] if len(segs) >= 2 else []),
                       inc='v')
                tseg = S('pe', (lambda e, mb=mb, c=c:
                                e.matmul(segps[:], rows16[:, mb, :],
                                         Mbuf[:, mb * R:(mb + 1) * R],
                                         start=(c == 0), stop=(c == AC - 1))),
                         waits=[trow, tM], inc='pe')
                segs.append(tseg)
            seg_state['last_segs'] = segs[-2:]
            return S('v', (lambda e, p=p:
                           e.tensor_copy(rbuf[:, p * R:(p + 1) * R], segps[:])),
                     waits=[tseg], inc='v')

        # --- L2 p1 ---
        y_cc0 = S('g', (lambda e: e.collective_compute(
                    'AllGather', mybir.AluOpType.bypass,
                    replica_groups=[list(range(NC))],
                    ins=[shardYs[0][:]], outs=[fullY[0][:]])),
                waits=[y_cc[0][2]], inc='cc')

        def mk_start_L2(waits, p=0):
            tok = None
            for nt in range(AC // 4):
                n0, n1 = nt * 512, (nt + 1) * 512
                tok = S('pe', (lambda e, n0=n0, n1=n1, p=p:
                               e.matmul(zpsum[:, n0:n1], wWsv[:],
                                        ybuf[:, p * LOCP + n0:p * LOCP + n1],
                                        start=True, stop=False)),
                        waits=[t_dma] + waits, inc='pe')
            return tok

        y_cc1 = S('g', (lambda e: e.collective_compute(
                    'AllGather', mybir.AluOpType.bypass,
                    replica_groups=[list(range(NC))],
                    ins=[shardYs[1][:]], outs=[fullY[1][:]])),
                waits=[y_cc[1][2]], inc='cc')
        lastpe1 = gathers_means(2, y_cc0, tabYrows[0], 1, 1,
                                ((lambda w: mk_start_L2(w, 0)),
                                 [tokens_y[1], rows_tok_y[1]]))
        t_w1 = S('act', lambda e: e.activation(wbuf[:], zpsum[:], AF.Relu),
                 waits=[lastpe1], inc='act')
        t_r[0] = emit_seg_chunks(0, t_w1)
        if DBG:
            t_dcp2 = S('v', (lambda e: e.tensor_copy(rT[:, 0:R], rbuf[:, 0:R])),
                       waits=[t_r[0]], inc='v')  # overwritten later by tm2
        w1_rows = [t for t in state['tr'] if t]
        tup1 = S('sync', (lambda e: e.dma_start(out=rparts_d[0][:],
                                                in_=rbuf[:, 0:R])),
                 waits=[t_r[0]], inc='dmaU', amt=16)

        # --- L2 p2 (gathers precede the p1 AllReduce in the Pool queue) ---
        lastpe2 = gathers_means(3, y_cc1, tabYrows[1], 2, 1,
                                ((lambda w: mk_start_L2(w, 1)),
                                 [tokens_y[1], t_w1, t_r[0]] + w1_rows))

        # --- rexchange p1 (AR fires after the L2p2 gathers, during p2 means) ---
        tcc1 = S('g', (lambda e: e.collective_compute(
                    'AllReduce', mybir.AluOpType.add,
                    replica_groups=[list(range(NC))],
                    ins=[rparts_d[0][:]], outs=[rsums_d[0][:]])),
                waits=[tup1], inc='cc')
        tdn1 = S('sync', (lambda e: e.dma_start(out=rbuf[:, 0:R],
                                                in_=rsums_d[0][:])),
                 waits=[tcc1], inc='dmaD1', amt=16)

        # --- rowsel / tsel / A (r1 recip folded into host sel matrix) ---
        # own psum tile (bank 7) so it never touches zpsum banks
        rowsel = []
        prev_cp = None
        for c in range(RC):
            n0 = c * 128
            nres = min((c + 1) * 128, R) - n0
            tk = S('pe', (lambda e, n0=n0, nres=nres:
                          e.transpose(rsp[0:nres, :], rbuf[:, n0:n0 + nres],
                                      iotaP[:])),
                   waits=[tdn1, t_id, prev_cp], inc='pe')
            prev_cp = S('act', (lambda e, c=c: e.activation(rowsR[:, c, :], rsp[:],
                                                            AF.Copy)),
                        waits=[tk], inc='act')
            rowsel.append(prev_cp)
        tsel = None
        for c in range(RC):
            nres = min((c + 1) * 128, R) - c * 128
            tsel = S('pe', (lambda e, c=c, nres=nres:
                            e.matmul(r1ps[:], rowsR[0:nres, c, :],
                                     wsel[0:nres, c * MYR:(c + 1) * MYR],
                                     start=(c == 0), stop=(c == RC - 1))),
                     waits=[rowsel[c], t_dma], inc='pe')
        t_r1my = S('act', lambda e: e.activation(r1my[:], r1ps[:], AF.Copy),
                   waits=[tsel], inc='act')

        tA = []
        for h in (0, 1):
            tk = S('pe', (lambda e, h=h:
                          e.matmul(Bps[h][:, 0:MYR], wWf1t[:, h * 128:(h + 1) * 128],
                                   r1my[:], start=True, stop=True)),
                   waits=[t_r1my, t_dma], inc='pe')
            tA.append(S('act', (lambda e, h=h: e.activation(
                            Abuf[:, h * MYR:(h + 1) * MYR], Bps[h][:, 0:MYR],
                            AF.Identity, bias=wbf1[:, h:h + 1])),
                        waits=[tk], inc='act'))

        t_w2 = S('act', lambda e: e.activation(wbuf[:], zpsum[:], AF.Relu),
                 waits=[lastpe2], inc='act')
        t_r[1] = emit_seg_chunks(1, t_w2)

        # --- rexchange p2 (full: AR + recip_res scale into rT) ---
        tup2 = S('sync', (lambda e: e.dma_start(out=rparts_d[1][:],
                                                in_=rbuf[:, R:2 * R])),
                 waits=[t_r[1]], inc='dmaU', amt=16)
        tcc2 = S('g', (lambda e: e.collective_compute(
                    'AllReduce', mybir.AluOpType.add,
                    replica_groups=[list(range(NC))],
                    ins=[rparts_d[1][:]], outs=[rsums_d[1][:]])),
                waits=[tup2], inc='cc')
        tdn2 = S('sync', (lambda e: e.dma_start(out=rbuf[:, R:2 * R],
                                                in_=rsums_d[1][:])),
                 waits=[tcc2], inc='dmaD2', amt=16)
        trr2 = S('pe', (lambda e: e.matmul(rrps[:], ones1[:],
                                           wrr[:, 0:R], start=True, stop=True)),
                 waits=[t_ones, t_dma, tdn2], inc='pe')
        tm2 = S('v', (lambda e: e.tensor_tensor(
                    out=rT[:, 0:R],
                    in0=rbuf[:, R:2 * R], in1=rrps[:],
                    op=AluOpType.mult)),
                waits=[trr2, tdn2], inc='v')

        # ---------- head: 2-row batches on 2 independent parity pipelines ----
        tB = []
        for h in (0, 1):
            tk = S('pe', (lambda e, h=h:
                          e.matmul(Bps[h][:], wWf1b[:, h * 128:(h + 1) * 128],
                                   rT[:, 0:R], start=True, stop=True)),
                   waits=[tm2] + tA, inc='pe')
            tB.append(S('v', (lambda e, h=h: e.tensor_copy(
                            Bbuf[:, h * R:(h + 1) * R], Bps[h][:])),
                        waits=[tk], inc='v'))

        prev_tmB = [None, None]
        prev_th2 = [None, None]
        prev_tm3 = [None, None]
        prev_tout = [None, None]
        outdma = [None] * 4
        NB = MYR // 2
        for b in range(NB):
            i0, par, slot = 2 * b, b % 2, b % 4
            txs = []
            for k in (0, 1):
                txs.append(S('v', (lambda e, i=i0 + k, k=k, par=par: e.tensor_scalar(
                            out=Xbuf[:, par, k * R:(k + 1) * R], in0=Bbuf[:, 0:R],
                            scalar1=Abuf[:, i:i + 1],
                            scalar2=0.0, op0=AluOpType.add, op1=AluOpType.max)),
                        waits=[tB[0], tA[0], prev_tmB[par]], inc='v'))
            for k in (0, 1):
                txs.append(S('act', (lambda e, i=i0 + k, k=k, par=par: e.activation(
                            Xbuf[:, par, (2 + k) * R:(3 + k) * R], Bbuf[:, R:2 * R],
                            AF.Relu, bias=Abuf[:, MYR + i:MYR + i + 1])),
                        waits=[tB[1], tA[1], prev_tmB[par]], inc='act'))
            for k in (0, 1):
                S('pe', (lambda e, par=par, k=k: e.matmul(
                            h2ps[par][:, k, 0:R], wWf2[:, 0:DF2],
                            Xbuf[:, par, k * R:(k + 1) * R],
                            start=True, stop=False)),
                  waits=[txs[k], prev_th2[par]], inc='pe')
            tmB_ = None
            for k in (0, 1):
                tmB_ = S('pe', (lambda e, par=par, k=k: e.matmul(
                            h2ps[par][:, k, 0:R], wWf2[:, DF2:2 * DF2],
                            Xbuf[:, par, (2 + k) * R:(3 + k) * R],
                            start=False, stop=True)),
                        waits=[txs[2 + k]], inc='pe')
            prev_tmB[par] = tmB_
            th2 = S('act', (lambda e, par=par: e.activation(
                        h2b[:, par, :].rearrange('p (two r) -> p two r', two=2),
                        h2ps[par][:, :, 0:R], AF.Relu, bias=wbf2[:])),
                    waits=[tmB_, prev_tm3[par]], inc='act')
            prev_th2[par] = th2
            tm3 = None
            for k in (0, 1):
                tm3 = S('pe', (lambda e, par=par, k=k: e.matmul(
                            h3ps[par][:, k, 0:R], wWf3[:],
                            h2b[:, par, k * R:(k + 1) * R],
                            start=True, stop=True)),
                       waits=[th2, prev_tout[par]], inc='pe')
            prev_tm3[par] = tm3
            t_out = S('v', (lambda e, par=par, slot=slot: e.tensor_scalar(
                          out=outb[:, slot, :].rearrange('o (two r) -> o two r', two=2),
                          in0=h3ps[par][:, :, 0:R],
                          scalar1=wbf3[:], scalar2=None, op0=AluOpType.add)),
                      waits=[tm3, outdma[slot]], inc='v')
            prev_tout[par] = t_out
            outdma[slot] = S('sync', D(out_ext[:, i0 * R:(i0 + 2) * R],
                                       outb[:, slot, :]),
                             waits=[t_out],
                             inc=('dmaR0', 'dmaR1', 'dmaO2', 'dmaO3')[slot], amt=16)
        state['outdma'] = [t for t in outdma if t]

        if DBG:
            S('sync', (lambda e: e.dma_start(out=dbg2_ext[:, 0:4096], in_=ybuf[:])),
              waits=state['outdma'], inc='dmaU', amt=16)
            S('sync', (lambda e: e.dma_start(out=dbg2_ext[:, 4096:4496], in_=rT[:, 0:R])),
              waits=[t_dcp2] + state['outdma'], inc='dmaU', amt=16)
            S('sync', (lambda e: e.dma_start(out=dbg_ext[:, 0:2 * R], in_=rbuf[:, 0:2 * R])),
              waits=[tdn2, t_r[1]] + state['outdma'], inc='dmaU', amt=16)
            S('sync', (lambda e: e.dma_start(out=dbg_ext[:, 2 * R:3 * R], in_=rT[:, 0:R])),
              waits=[tm2] + state['outdma'], inc='dmaU', amt=16)
            S('sync', (lambda e: e.dma_start(out=dbg_ext[:, 3 * R:3 * R + MYR], in_=r1my[:])),
              waits=[t_r1my] + state['outdma'], inc='dmaU', amt=16)
            S('sync', (lambda e: e.dma_start(out=dbg_ext[:, 3 * R + MYR:3 * R + 3 * MYR], in_=Abuf[:])),
              waits=tA + state['outdma'], inc='dmaU', amt=16)
            S('sync', (lambda e: e.dma_start(
                  out=dbg_ext[:, 3 * R + 3 * MYR:3 * R + 3 * MYR + 4 * 64],
                  in_=rowsR[:].bitcast(mybir.dt.float32).rearrange('p c e -> p (c e)'))),
              waits=state['outdma'], inc='dmaU', amt=16)
        S('sync', lambda e: e.nop(), waits=state['outdma'])

        @block.sync
        def _(e):
            for eng, fn in steps:
                if eng == 'sync':
                    fn(e)

        @block.tensor
        def _(e):
            for eng, fn in steps:
                if eng == 'pe':
                    fn(e)

        @block.vector
        def _(e):
            for eng, fn in steps:
                if eng == 'v':
                    fn(e)

        @block.scalar
        def _(e):
            for eng, fn in steps:
                if eng == 'act':
                    fn(e)

        @block.gpsimd
        def _(e):
            for eng, fn in steps:
                if eng == 'g':
                    fn(e)

    nc.finalize()
    return nc


def _bf(x):
    import ml_dtypes
    return np.asarray(x, np.float32).astype(ml_dtypes.bfloat16)


def prep_inputs(inputs, N, R, BERT):
    LOC = N // NC
    LOCP = -(-LOC // 128) * 128
    STR = LOCP // 128 + (1 if LOC == LOCP else 0)
    KB = BERT // 128
    MYR = R // NC
    RC = -(-R // 128)
    AC = LOCP // 128
    CH_AT = 256
    CH = CH_AT * K
    BL = CH // 128
    NSEG = LOCP // CH_AT
    RPC = 2 * NSEG * BL
    f32 = np.float32
    NID = 2 * LOCP * K

    zero_loc = LOC % 128 + (LOC // 128) * 128 if LOC < LOCP else (STR - 1) * 128

    def wrap_idx(flat):
        nid = flat.shape[0]
        w = flat.reshape(nid // 16, 16).T.astype(np.int16)
        return np.tile(w, (8, 1))

    def mk_idx(same, diff, core, layer):
        lo = core * LOC
        parts = []
        for tab, idx in ((0, same), (1, diff)):
            sl = np.asarray(idx)[lo:lo + LOC].astype(np.int64)
            rank, locl = sl // LOC, sl % LOC
            if layer == 0:
                base = (rank * 2 + tab) * STR * 128
                zs = tab * STR * 128 + zero_loc
            else:
                base = rank * STR * 128
                zs = zero_loc
            s = np.where(sl < 0, zs, base + locl)
            # pad atoms' indices are -1: they are the trailing indices of the
            # side's tail gather chunk, trimmed by the ucode (num_idxs_reg).
            pad = np.full((LOCP - LOC, K), zs, np.int64)
            s = np.concatenate([s, pad], 0).reshape(-1)
            parts.append(s)
        flat = np.concatenate(parts)
        assert flat.max() < 32768, flat.max()
        return wrap_idx(flat)

    def mk_recip_pos(same, diff, core):
        lo = core * LOC
        out = np.zeros((128, RPC), f32)
        pos = np.arange(CH)
        atom_l = pos // K          # atom within chunk [0, CH_AT)
        for t, idx in ((0, same), (1, diff)):
            m = (np.asarray(idx)[lo:lo + LOC] > -1).sum(1)
            recip = np.zeros(LOCP, f32)
            recip[:LOC] = 1.0 / np.maximum(m, 1)
            for q in range(NSEG):
                vals = recip[q * CH_AT + atom_l]          # [CH]
                cols = vals.reshape(BL, 128).T            # [128, BL]
                out[:, (t * NSEG + q) * BL:(t * NSEG + q + 1) * BL] = cols
        return out

    cnt_res = [np.zeros(R, f32), np.zeros(R, f32)]
    for p, rid in ((0, inputs['res_ids1']), (1, inputs['res_ids2'])):
        ids, c = np.unique(np.asarray(rid), return_counts=True)
        cnt_res[p][ids.astype(int)] = c
    recip_res_v = [1.0 / np.maximum(cnt_res[0], 1), 1.0 / np.maximum(cnt_res[1], 1)]
    recip_res = np.concatenate(recip_res_v).reshape(1, 2 * R).astype(f32)

    # static one-hot K-window matrices: oneH[p, j*128 + a] = ((j*128+p)//K == a)
    oneH = np.zeros((128, K * 128), f32)
    for j in range(K):
        p_ = np.arange(128)
        a_ = (j * 128 + p_) // K
        oneH[p_, j * 128 + a_] = 1.0

    Wf1 = np.asarray(inputs['Wf1'], f32)
    Wf2 = np.asarray(inputs['Wf2'], f32)
    shared = {
        'Wv': _bf(inputs['Wv']),
        'Wr': _bf(np.asarray(inputs['Wr'], f32).reshape(KB, 128, 128).transpose(1, 0, 2).reshape(128, KB * 128)),
        'Wsr1': _bf(inputs['Wsr1']), 'Wdr1': _bf(inputs['Wdr1']),
        'Wsv': np.asarray(inputs['Wsv'], f32),
        'Wsr2': np.asarray(inputs['Wsr2'], np.float32),
        'Wdr2': np.asarray(inputs['Wdr2'], np.float32),
        'Wf1t': Wf1[:128, :], 'Wf1b': Wf1[128:, :],
        'Wf2': _bf(np.concatenate([Wf2[:128], Wf2[128:]], axis=1)),
        'Wf3': _bf(np.asarray(inputs['Wf3'], f32).reshape(DF2, 1)),
        'bf1': np.asarray(inputs['bf1'], f32).reshape(2, 128).T.copy(),
        'bf2': np.asarray(inputs['bf2'], f32).reshape(DF2, 1),
        'bf3': np.asarray(inputs['bf3'], f32).reshape(1, 1),
        'recip_res': recip_res,
        'oneH': _bf(oneH),
    }
    per_core = []
    for core in range(NC):
        m = dict(shared)
        lo = core * LOC
        for p, (a, r, s, d, rid) in enumerate((
                ('atoms1', 'residues1', 'same1', 'diff1', 'res_ids1'),
                ('atoms2', 'residues2', 'same2', 'diff2', 'res_ids2'))):
            at = np.zeros((ATOM_CAT, LOCP), f32)
            at[:, :LOC] = np.asarray(inputs[a], f32)[lo:lo + LOC].T
            m[f'atomsT_{p + 1}'] = _bf(at)
            rt = np.zeros((BERT, LOCP), f32)
            rt[:, :LOC] = np.asarray(inputs[r], f32)[lo:lo + LOC].T
            m[f'residT_{p + 1}'] = _bf(rt.reshape(KB, 128, LOCP))
            m[f'idxL1_{p + 1}'] = mk_idx(inputs[s], inputs[d], core, 0)
            m[f'idxL2_{p + 1}'] = mk_idx(inputs[s], inputs[d], core, 1)
            m[f'recipPos_{p + 1}'] = _bf(mk_recip_pos(inputs[s], inputs[d], core))
            rr = np.full((LOCP,), -1.0, f32)
            rr[:LOC] = np.asarray(inputs[rid], f32)[lo:lo + LOC]
            m[f'rids_{p + 1}'] = rr.reshape(AC, 128).T.copy()
        # row-select matrix with recip_res1 folded in (r1 residue-mean scale)
        sel = np.zeros((128, RC * MYR), f32)
        for j in range(MYR):
            g = core * MYR + j
            sel[g % 128, (g // 128) * MYR + j] = recip_res_v[0][g]
        m['sel'] = _bf(sel)
        per_core.append(m)
    return per_core


def kernel(**inputs):
    from concourse.bass_utils import run_bass_kernel_spmd
    nc = build_graph(N_ATOMS, N_RES, BERT_DIM)
    in_maps = prep_inputs(inputs, N_ATOMS, N_RES, BERT_DIM)
    res = run_bass_kernel_spmd(nc, in_maps, list(range(NC)))
    out = np.concatenate([np.asarray(res.results[c]['out']).reshape(-1) for c in range(NC)])
    return out.astype(np.float32)

